# revision 13
# baseline (speedup 1.0000x reference)
"""Conv2DMod Trainium2 kernel, plan C: F(4,3) along H x F(2,3) along W.

Problem: B=8, C_in=512, C_out=512, K=3x3, H=W=64, fp32, 'same' padding.
One sample per NeuronCore (8 cores).

Byte-count-optimized hybrid 2D Winograd (the plan-B F(4,3)^2 kernel is
HBM-bound at 37.7MB/core/rep; this variant needs 31.5MB):

  H axis: F(4,3), points {0,1,-1,1/2,-2}, 6 coords 'a'. Forward on
          host; INVERSE ON HOST (M leaves the device in A^T-w-reduced
          form only).
  W axis: F(2,3), points {0,1,-1}, 4 coords 'b'. Forward on host;
          inverse ON DEVICE: all 4 jw coords of a group live in the
          same PSUM tile, so z0 = m0+m1+m2, z1 = m1-m2+m3 costs just
          4 fp16 DVE ops per group - no cross-group staging.

  device per (a, ot) group:  16 MMs (4 jw x 4 ct), free dim 512
     M[b][o, th, tw] = sum_c W2[a,b,c,o] U2[c,a,b,th,tw]   (PSUM)
     drain c[0:3] on ACT, c[3] on DVE (fp16), inverse-W on DVE,
     z -> HBM via the gpsimd SWDGE ring.

DMA/core/rep: U 12.6MB + W 12.6MB + z-out 6.3MB = 31.5MB (~90us at the
~350GB/s per-core share of HBM).  PE: 384 MMs x ~216ns = 83us.
Measured end-to-end rel err ~2.6e-3.
"""

import contextlib

import numpy as np

import concourse.bass as bass
import concourse.tile as tile
from concourse import bacc, mybir
from concourse.bass_utils import run_bass_kernel_spmd

B = 8
C = 512
O = 512
H = W = 64
A6 = 6           # H-axis Winograd coords
JW = 4           # W-axis Winograd coords
TH = 16          # H tiles (64 / 4)
TW = 32          # W tiles (64 / 2)
NT = TH * TW     # 512 -> matmul free dim
CT = 4
OT = 4
EPS = 1e-8

F16 = mybir.dt.float16
F32 = mybir.dt.float32


def _cook_toom(pts, m, r=3):
    n = m + r - 1
    a = pts
    AT = np.zeros((m, n))
    for i in range(m):
        for j in range(n - 1):
            AT[i, j] = a[j] ** i
    AT[m - 1, n - 1] = 1.0
    G = np.zeros((n, r))
    for i in range(n - 1):
        c = np.prod([a[i] - a[k] for k in range(n - 1) if k != i])
        for j in range(r):
            G[i, j] = a[i] ** j / c
    G[n - 1, r - 1] = 1.0
    rows, rhs = [], []
    for wi in range(r):
        w = np.zeros(r)
        w[wi] = 1
        gw = G @ w
        for di in range(n):
            for k in range(m):
                row = np.zeros((n, n))
                for j in range(n):
                    row[j, di] = AT[k, j] * gw[j]
                rows.append(row.ravel())
                rhs.append(w[di - k] if 0 <= di - k < r else 0.0)
    BTf, _, _, _ = np.linalg.lstsq(np.array(rows), np.array(rhs), rcond=None)
    assert np.abs(np.array(rows) @ BTf - np.array(rhs)).max() < 1e-9
    return AT, G, BTf.reshape(n, n)


ATH, GH, BTH = _cook_toom([0, 1, -1, 0.5, -2], m=4)
ATW, GW, BTW = _cook_toom([0, 1, -1], m=2)


def build_nc(reps=1):
    nc = bacc.Bacc(None, target_bir_lowering=False)

    # Partition-major layouts: one DMA per logical transfer.
    # Weights ship H-transformed only (W_h = G_h w, 3 kw taps: 9.4MB
    # instead of 12.6MB); the W-axis G_w transform happens on-device:
    # with G_w = [[-1,0,0],[.5,.5,.5],[.5,-.5,.5],[0,0,1]], coords b0/b3
    # are just (sign/scale-folded) raw slices and only b1/b2 = s +- w1
    # (s = w0+w2) need 3 DVE ops per (a,ot) group.  The -1 and 0.5
    # factors are folded into U2 on the host.
    u_d = nc.dram_tensor("u", [A6, 128, CT, JW * NT], F16,
                         kind="ExternalInput")
    w_d = nc.dram_tensor("w", [A6, OT, 128, CT * 3 * 128], F16,
                         kind="ExternalInput")
    z_d = nc.dram_tensor("z", [A6, O, 2, NT], F16, kind="ExternalOutput")

    with tile.TileContext(nc) as tc:
      for _rep in range(reps):
        with contextlib.ExitStack() as stack:
            # Double-buffer the LAST-consumed U tile (a=5): with bufs=1
            # its next-rep DMA would WAR-wait on this rep's final matmul
            # block, serializing ~12us of U reload at every rep boundary.
            # The other tiles' consumers finish early enough that bufs=1
            # already lets their next-rep DMAs stream in mid-rep.
            u_pools = [
                stack.enter_context(tc.tile_pool(
                    name=f"u{a}", bufs=(2 if a == A6 - 1 else 1)))
                for a in range(A6)
            ]
            w_pool = stack.enter_context(tc.tile_pool(name="ws", bufs=10))
            wt_pool = stack.enter_context(tc.tile_pool(name="wt", bufs=4))
            ws_pool = stack.enter_context(tc.tile_pool(name="wss", bufs=4))
            c_pool = stack.enter_context(tc.tile_pool(name="cs", bufs=4))
            t_pool = stack.enter_context(tc.tile_pool(name="ts", bufs=4))
            z_pool = stack.enter_context(tc.tile_pool(name="zs", bufs=4))
            m_pool = stack.enter_context(tc.tile_pool(
                name="mp", bufs=2, space=bass.MemorySpace.PSUM))

            # U and W stream on the SP HWDGE ring; z-out on the gpsimd
            # SWDGE ring (never head-of-line blocks U/W).
            uts = []
            for a in range(A6):
                ut = u_pools[a].tile([128, CT, JW, NT], F16,
                                     name=f"u{a}", tag=f"u{a}")
                nc.sync.dma_start(ut[:], u_d[a])
                uts.append(ut)

            for a in range(A6):
                for ot in range(OT):
                    o0 = ot * 128
                    wh = w_pool.tile([128, CT, 3, 128], F16,
                                     name="w", tag="w")
                    nc.sync.dma_start(wh[:], w_d[a, ot])
                    # On-device G_w: s = w0+w2; b1 = s+w1; b2 = s-w1.
                    st = ws_pool.tile([128, CT, 128], F16, name="s")
                    w12 = wt_pool.tile([128, CT, 2, 128], F16, name="w12")
                    nc.vector.tensor_add(st, wh[:, :, 0], wh[:, :, 2])
                    nc.vector.tensor_add(w12[:, :, 0], st, wh[:, :, 1])
                    nc.vector.tensor_sub(w12[:, :, 1], st, wh[:, :, 1])
                    mt = m_pool.tile([128, JW, NT], F32, name="mt")
                    for b in range(JW):
                        for ct in range(CT):
                            lhs = (wh[:, ct, 0, :] if b == 0 else
                                   w12[:, ct, 0, :] if b == 1 else
                                   w12[:, ct, 1, :] if b == 2 else
                                   wh[:, ct, 2, :])
                            nc.tensor.matmul(
                                mt[:, b],
                                lhs,
                                uts[a][:, ct, b, :],
                                start=(ct == 0),
                                stop=(ct == CT - 1),
                            )
                    # Drain PSUM -> fp16 wholly on ACT (DVE carries the
                    # weight transform + inverse).
                    ctile = c_pool.tile([128, JW, NT], F16, name="c")
                    nc.scalar.copy(ctile[:], mt[:])
                    # Inverse W-transform (F(2,3), points {0,1,-1}):
                    #   z0 = m0 + m1 + m2 ; z1 = m1 - m2 + m3
                    ztile = z_pool.tile([128, 2, NT], F16, name="z")
                    t01 = t_pool.tile([128, NT], F16, name="t01")
                    s12 = t_pool.tile([128, NT], F16, name="s12")
                    nc.vector.tensor_add(t01, ctile[:, 0], ctile[:, 1])
                    nc.vector.tensor_add(ztile[:, 0], t01, ctile[:, 2])
                    nc.vector.tensor_sub(s12, ctile[:, 1], ctile[:, 2])
                    nc.vector.tensor_add(ztile[:, 1], s12, ctile[:, 3])
                    nc.gpsimd.dma_start(z_d[a, o0:o0 + 128], ztile[:])

    nc.compile()
    return nc


def prep_inputs(x, y, weight):
    """Host: modulation+demod fold + forward Winograd transforms."""
    x = np.asarray(x, dtype=np.float32)
    y = np.asarray(y, dtype=np.float32)
    weight = np.asarray(weight, dtype=np.float32)

    s = y + 1.0
    wts = weight[None] * s[:, None, :, None, None]
    d = 1.0 / np.sqrt((wts * wts).sum(axis=(2, 3, 4), keepdims=True) + EPS)
    wmod = (wts * d).astype(np.float64)             # [B, O, C, 3, 3]

    rh = 4 * np.arange(TH)[:, None] + np.arange(A6)[None, :]   # [16, 6]
    rw = 2 * np.arange(TW)[:, None] + np.arange(JW)[None, :]   # [32, 4]

    # Per-b folds compensating the device-side G_w form (b0 uses +w0 for
    # true weight -w0; device b1/b2 = s+-w1 are 2x the true G_w rows).
    fold = np.array([-1.0, 0.5, 0.5, 1.0])[None, :, None, None]

    in_maps = []
    for b in range(B):
        # H-transformed weights only: W_h[a, kw, c, o] = sum_kh GH wmod.
        whst = np.einsum("ak,ockl->alco", GH, wmod[b])       # [6,3,C,O]
        whst = whst.reshape(A6, 3, CT, 128, OT, 128).transpose(
            0, 4, 3, 2, 1, 5)
        whst = np.ascontiguousarray(
            whst.reshape(A6, OT, 128, CT * 3 * 128)).astype(np.float16)

        xp = np.zeros((C, H + 2, W + 2), np.float32)
        xp[:, 1:-1, 1:-1] = x[b]
        win = xp[:, rh[:, :, None, None], rw[None, None, :, :]]
        # win: [C, TH, 6i, TW, 4j]
        u2 = np.einsum("ai,ctiuj,bj->cabtu", BTH, win.astype(np.float64),
                       BTW)
        u2 = u2 * fold[None]
        u2 = u2.reshape(CT, 128, A6, JW, NT).transpose(2, 1, 0, 3, 4)
        u2 = np.ascontiguousarray(
            u2.reshape(A6, 128, CT, JW * NT)).astype(np.float16)
        in_maps.append({"u": u2, "w": whst})
    return in_maps


def finish_output(res_list):
    """Host inverse H-transform: out[o, 4t+r, 2u+s] = sum_a ATH[r,a] z."""
    at32 = ATH.astype(np.float32)
    outs = []
    for r in res_list:
        z = r["z"].astype(np.float32).reshape(A6, O, 2, TH, TW)
        out = np.einsum("ra,aowtu->otruw", at32, z)  # [O, TH, 4, TW, 2]
        outs.append(np.ascontiguousarray(out.reshape(O, H, W)))
    return np.stack(outs, axis=0)


OUT_TENSOR = "z"

_CACHE = {}


def _get_nc():
    if "nc" not in _CACHE:
        _CACHE["nc"] = build_nc()
    return _CACHE["nc"]


def kernel(x, y, weight):
    in_maps = prep_inputs(x, y, weight)
    nc = _get_nc()
    res = run_bass_kernel_spmd(nc, in_maps, core_ids=list(range(B)))
    kernel.last_results = res
    return finish_output(res.results)


kernel.last_results = None
